# revision 3
# baseline (speedup 1.0000x reference)
"""Distributed Trainium2 kernel for causal multi-head attention with RoPE.

Problem (hardcoded): B=2, S=2048, D=2048, H=16, DH=128, float32 I/O.
  out = softmax(mask + rope(x@wq.T) @ rope(x@wk.T).T / sqrt(DH)) @ (x@wv.T) @ wo.T

Sharding over 8 NeuronCores: batch (2) x head-group (4).
Core c handles batch b=c//4 and heads [4g, 4g+4) with g=c%4:
  - QKV projections computed in transposed layout qT/kT [d, tok] (bf16 compute,
    f32 accumulation in PSUM); v in [tok, d] layout.
  - RoPE applied in transposed layout: rot = qT*C + pairswap(qT)*S, where the
    pair swap is a PE matmul with a permutation matrix and C/S are host-built
    [128, 2048] matrices from freqs_cos/sin. 1/sqrt(DH) is folded into wq.
  - Causal attention per head, dual score layouts:
      stats path  [q, k]: exp with accum_out -> row sums -> 1/r broadcast
      AV path     [k, q]: exp tiles (masked) feed attn@V directly, no transpose
    (softmax max-subtraction skipped: scores are O(3), exp cannot overflow)
  - AllGather of per-head attention outputs avT over the 4-core batch group.
  - Output projection is column-parallel: each core computes all 2048 tokens of
    its batch for its 512 output columns (woT host-sliced per core).
Host: shards/prepares inputs per core, runs one SPMD NEFF on cores 0-7,
assembles out[b, :, 512g:512(g+1)] from each core.
"""

import sys

for _p in ("/opt/trn_rl_repo", "/root/.axon_site/_ro/trn_rl_repo"):
    if _p not in sys.path:
        sys.path.insert(0, _p)

import math
import numpy as np
import ml_dtypes

import concourse.bass as bass
import concourse.bacc as bacc
import concourse.mybir as mybir
from concourse import tile
from concourse.bass_utils import run_bass_kernel_spmd

bf16 = ml_dtypes.bfloat16
F32 = mybir.dt.float32
BF16 = mybir.dt.bfloat16
Exp = mybir.ActivationFunctionType.Exp
AX = mybir.AxisListType.X
ADD = mybir.AluOpType.add

B, S, D, H = 2, 2048, 2048, 16
DH = D // H  # 128
HPC = 4  # heads per core
GROUPS = [[0, 1, 2, 3], [4, 5, 6, 7]]
NIC = D // 128  # 16 contraction chunks
NTB = S // 512  # 4 token blocks of 512
NTC = S // 128  # 16 token chunks of 128

_GRAPH_CACHE = {}


def build_graph():
    if "nc" in _GRAPH_CACHE:
        return _GRAPH_CACHE["nc"]
    nc = bacc.Bacc(None)

    xT_d = nc.declare_dram_parameter("xT", [D, S], BF16, isOutput=False)
    wqT_d = nc.declare_dram_parameter("wqT", [D, 512], BF16, isOutput=False)
    wkT_d = nc.declare_dram_parameter("wkT", [D, 512], BF16, isOutput=False)
    wvT_d = nc.declare_dram_parameter("wvT", [D, 512], BF16, isOutput=False)
    woT_d = nc.declare_dram_parameter("woT", [D, 512], BF16, isOutput=False)
    cmat_d = nc.declare_dram_parameter("cmat", [128, S], F32, isOutput=False)
    smat_d = nc.declare_dram_parameter("smat", [128, S], F32, isOutput=False)
    pmat_d = nc.declare_dram_parameter("pmat", [128, 128], BF16, isOutput=False)
    mmul_d = nc.declare_dram_parameter("mmul", [128, 2048], BF16, isOutput=False)
    mstat_d = nc.declare_dram_parameter("mstat", [128, 2048], BF16, isOutput=False)
    out_d = nc.declare_dram_parameter("out", [S, 512], F32, isOutput=True)

    ag_in = nc.dram_tensor("ag_in", [512, S], BF16)
    ag_out = nc.dram_tensor("ag_out", [D, S], BF16)
    r_bounce = nc.dram_tensor("r_bounce", [HPC, S], F32)

    with tile.TileContext(nc) as tc:
        with tc.tile_pool(name="work", bufs=2) as wk:
            with tc.tile_pool(name="poolA", bufs=1) as pa:
                # persistent across QKV + attention
                qrot = [pa.tile([128, S], BF16, tag=f"q{h}", name=f"qrot{h}") for h in range(HPC)]
                krot = [pa.tile([128, S], BF16, tag=f"k{h}", name=f"krot{h}") for h in range(HPC)]
                vsb = [pa.tile([128, 512], BF16, tag=f"v{j}", name=f"vsb{j}") for j in range(NTC)]

                # ============ Stage 1+2: QKV projections + RoPE =============
                with (
                    tc.tile_pool(name="qkvw", bufs=1) as qw,
                    tc.tile_pool(name="psq", bufs=4, space="PSUM") as psq,
                    tc.tile_pool(name="pssh", bufs=2, space="PSUM") as pssh,
                    tc.tile_pool(name="psv", bufs=2, space="PSUM") as psv,
                ):
                    xt = qw.tile([128, NIC * S], BF16, tag="xt")
                    for i in range(NIC):
                        nc.sync.dma_start(xt[:, S * i : S * (i + 1)], xT_d[128 * i : 128 * (i + 1), :])
                    wq_sb = qw.tile([128, NIC * 512], BF16, tag="wq")
                    wk_sb = qw.tile([128, NIC * 512], BF16, tag="wk")
                    cs_sb = qw.tile([128, S], F32, tag="cs")
                    sn_sb = qw.tile([128, S], F32, tag="sn")
                    pmat_sb = qw.tile([128, 128], BF16, tag="pmat")
                    for i in range(NIC):
                        nc.sync.dma_start(wq_sb[:, 512 * i : 512 * (i + 1)], wqT_d[128 * i : 128 * (i + 1), :])
                        nc.sync.dma_start(wk_sb[:, 512 * i : 512 * (i + 1)], wkT_d[128 * i : 128 * (i + 1), :])
                    nc.sync.dma_start(cs_sb[:], cmat_d[:])
                    nc.sync.dma_start(sn_sb[:], smat_d[:])
                    nc.sync.dma_start(pmat_sb[:], pmat_d[:])

                    # Q and K projections -> transposed layout [d, tok] + RoPE
                    for w_sb, rot in ((wq_sb, qrot), (wk_sb, krot)):
                        for h in range(HPC):
                            for b in range(NTB):
                                ps = psq.tile([128, 512], F32, tag="qk")
                                for i in range(NIC):
                                    nc.tensor.matmul(
                                        ps[:],
                                        w_sb[:, 512 * i + 128 * h : 512 * i + 128 * (h + 1)],
                                        xt[:, S * i + 512 * b : S * i + 512 * (b + 1)],
                                        start=(i == 0),
                                        stop=(i == NIC - 1),
                                    )
                                raw = wk.tile([128, 512], BF16, tag="raw")
                                nc.scalar.copy(raw[:], ps[:])
                                shp = pssh.tile([128, 512], F32, tag="sh")
                                nc.tensor.matmul(shp[:], pmat_sb[:], raw[:])
                                t1 = wk.tile([128, 512], F32, tag="t1")
                                t2 = wk.tile([128, 512], F32, tag="t2")
                                nc.vector.tensor_mul(t1[:], ps[:], cs_sb[:, 512 * b : 512 * (b + 1)])
                                nc.vector.tensor_mul(t2[:], shp[:], sn_sb[:, 512 * b : 512 * (b + 1)])
                                nc.vector.tensor_add(rot[h][:, 512 * b : 512 * (b + 1)], t1[:], t2[:])

                    # V projection -> [tok, d] layout, streamed wv chunks
                    for j in range(NTC):
                        ps = psv.tile([128, 512], F32, tag="v")
                        for i in range(NIC):
                            wv_t = wk.tile([128, 512], BF16, tag="wv")
                            nc.sync.dma_start(wv_t[:], wvT_d[128 * i : 128 * (i + 1), :])
                            nc.tensor.matmul(
                                ps[:],
                                xt[:, S * i + 128 * j : S * i + 128 * (j + 1)],
                                wv_t[:],
                                start=(i == 0),
                                stop=(i == NIC - 1),
                            )
                        nc.scalar.copy(vsb[j][:], ps[:])

                # ============ Stage 3: attention per head ===================
                with (
                    tc.tile_pool(name="attn", bufs=2) as at,
                    tc.tile_pool(name="attn1", bufs=1) as at1,
                    tc.tile_pool(name="psa", bufs=2, space="PSUM") as psa,
                    tc.tile_pool(name="psb", bufs=2, space="PSUM") as psb,
                    tc.tile_pool(name="psav", bufs=2, space="PSUM") as psav,
                    tc.tile_pool(name="psr", bufs=2, space="PSUM") as psr,
                ):
                    avsb = [at1.tile([128, S], BF16, tag=f"av{h}", name=f"avsb{h}") for h in range(HPC)]
                    mmul_sb = at1.tile([128, 2048], BF16, tag="mmul")
                    mstat_sb = at1.tile([128, 2048], BF16, tag="mstat")
                    ones_sb = at1.tile([1, 128], F32, tag="ones")
                    nc.vector.memset(ones_sb[:], 1.0)
                    nc.sync.dma_start(mmul_sb[:], mmul_d[:])
                    nc.sync.dma_start(mstat_sb[:], mstat_d[:])

                    for h in range(HPC):
                        # ---- stats: r[q] = sum over valid k of exp(s[q,k]) ----
                        rall = at.tile([128, NTC], F32, tag="rall")
                        for qb in range(NTC):
                            q0 = 128 * qb
                            nk = qb // 4 + 1
                            racc = wk.tile([128, 4], F32, tag="racc")
                            for j in range(nk):
                                ps = psa.tile([128, 512], F32, tag="sa")
                                nc.tensor.matmul(
                                    ps[:],
                                    qrot[h][:, q0 : q0 + 128],
                                    krot[h][:, 512 * j : 512 * (j + 1)],
                                )
                                es = wk.tile([128, 512], BF16, tag="es")
                                if j == nk - 1:
                                    # diagonal-band tile: exp, mask, reduce
                                    nc.scalar.activation(es[:], ps[:], Exp)
                                    em = wk.tile([128, 512], BF16, tag="em")
                                    moff = 512 * (qb % 4)
                                    nc.vector.tensor_mul(em[:], es[:], mstat_sb[:, moff : moff + 512])
                                    nc.vector.tensor_reduce(racc[:, j : j + 1], em[:], AX, ADD)
                                else:
                                    nc.scalar.activation(es[:], ps[:], Exp, accum_out=racc[:, j : j + 1])
                            nc.vector.tensor_reduce(rall[:, qb : qb + 1], racc[:, :nk], AX, ADD)
                        # 1/r, transpose to [1, S] via DRAM bounce, PE broadcast
                        rinv = at.tile([128, NTC], F32, tag="rinv")
                        nc.vector.reciprocal(rinv[:], rall[:])
                        nc.sync.dma_start(
                            r_bounce[h].rearrange("(b a) -> a b", a=128, b=NTC), rinv[:]
                        )
                        rT = at.tile([1, S], F32, tag="rT")
                        nc.sync.dma_start(rT[0:1, :], r_bounce[h][None, :])
                        rbc = at.tile([128, S], F32, tag="rbc")
                        for b in range(NTB):
                            ps = psr.tile([128, 512], F32, tag="rb")
                            nc.tensor.matmul(ps[:], ones_sb[:], rT[0:1, 512 * b : 512 * (b + 1)])
                            nc.scalar.copy(rbc[:, 512 * b : 512 * (b + 1)], ps[:])

                        # ---- AV path: avT[d, q] accumulation over k chunks ----
                        for b in range(NTB):
                            q0 = 512 * b
                            nk2 = 4 * (b + 1)
                            av = psav.tile([128, 512], F32, tag="av")
                            for kc in range(nk2):
                                ps = psb.tile([128, 512], F32, tag="sb")
                                nc.tensor.matmul(
                                    ps[:],
                                    krot[h][:, 128 * kc : 128 * (kc + 1)],
                                    qrot[h][:, q0 : q0 + 512],
                                )
                                et = at.tile([128, 512], BF16, tag="et")
                                nc.scalar.activation(et[:], ps[:], Exp)
                                if kc >= 4 * b:
                                    j = kc - 4 * b
                                    nc.vector.tensor_mul(et[:], et[:], mmul_sb[:, 512 * j : 512 * (j + 1)])
                                nc.tensor.matmul(
                                    av[:],
                                    vsb[kc][:, 128 * h : 128 * (h + 1)],
                                    et[:],
                                    start=(kc == 0),
                                    stop=(kc == nk2 - 1),
                                )
                            nc.vector.tensor_mul(
                                avsb[h][:, q0 : q0 + 512], av[:], rbc[:, q0 : q0 + 512]
                            )

                    # send local heads to the batch group
                    for h in range(HPC):
                        nc.sync.dma_start(ag_in[128 * h : 128 * (h + 1), :], avsb[h][:])
                    nc.gpsimd.collective_compute(
                        "AllGather",
                        mybir.AluOpType.bypass,
                        replica_groups=GROUPS,
                        ins=[ag_in[:]],
                        outs=[ag_out[:]],
                    )

            # ============ Stage 4: column-parallel wo projection ============
            with (
                tc.tile_pool(name="wop", bufs=1) as wo,
                tc.tile_pool(name="pswo", bufs=4, space="PSUM") as pswo,
            ):
                ag_sb = [wo.tile([128, S], BF16, tag=f"ag{cc}", name=f"agsb{cc}") for cc in range(NIC)]
                wo_sb = [wo.tile([128, 512], BF16, tag=f"wo{cc}", name=f"wosb{cc}") for cc in range(NIC)]
                for cc in range(NIC):
                    nc.sync.dma_start(ag_sb[cc][:], ag_out[128 * cc : 128 * (cc + 1), :])
                    nc.sync.dma_start(wo_sb[cc][:], woT_d[128 * cc : 128 * (cc + 1), :])
                for t in range(NTC):
                    ps = pswo.tile([128, 512], F32, tag="wo")
                    for cc in range(NIC):
                        nc.tensor.matmul(
                            ps[:],
                            ag_sb[cc][:, 128 * t : 128 * (t + 1)],
                            wo_sb[cc][:],
                            start=(cc == 0),
                            stop=(cc == NIC - 1),
                        )
                    osb = wk.tile([128, 512], F32, tag="osb")
                    nc.vector.tensor_copy(osb[:], ps[:])
                    nc.sync.dma_start(out_d[128 * t : 128 * (t + 1), :], osb[:])

    nc.finalize()
    _GRAPH_CACHE["nc"] = nc
    return nc


def _host_prep(x, freqs_cos, freqs_sin, wq, wk, wv, wo):
    """Build the 8 per-core input maps."""
    fc = np.asarray(freqs_cos, np.float32)  # [S, 64]
    fs = np.asarray(freqs_sin, np.float32)
    cmat = np.empty((128, S), np.float32)
    smat = np.empty((128, S), np.float32)
    cmat[0::2, :] = fc.T[:, :]  # row 2i   <- cos[:, i]
    cmat[1::2, :] = fc.T[:, :]
    smat[0::2, :] = -fs.T[:, :]  # rot[2i]   = a*c - b*s ; shuf[2i]   = b
    smat[1::2, :] = fs.T[:, :]  # rot[2i+1] = b*c + a*s ; shuf[2i+1] = a
    pmat = np.zeros((128, 128), np.float32)
    for i in range(64):
        pmat[2 * i, 2 * i + 1] = 1.0  # shuf = P @ q, P symmetric pair swap
        pmat[2 * i + 1, 2 * i] = 1.0

    xs = np.arange(128)[:, None]
    ys = np.arange(512)[None, :]
    # AV-path masks, [128 k x 512 q] tiles: valid iff x + 128*j <= y
    mm = np.zeros((128, 4, 512), np.float32)
    # stats-path masks, [128 q x 512 k] tiles: valid iff y <= x + 128*o
    ms = np.zeros((128, 4, 512), np.float32)
    for j in range(4):
        mm[:, j, :] = (xs + 128 * j <= ys).astype(np.float32)
        ms[:, j, :] = (ys <= xs + 128 * j).astype(np.float32)
    mmul = mm.reshape(128, 2048)
    mstat = ms.reshape(128, 2048)

    wq_s = np.asarray(wq, np.float32) / math.sqrt(DH)
    wk_s = np.asarray(wk, np.float32)
    wv_s = np.asarray(wv, np.float32)
    wo_s = np.asarray(wo, np.float32)
    x = np.asarray(x, np.float32)

    shared = {
        "cmat": cmat,
        "smat": smat,
        "pmat": pmat.astype(bf16),
        "mmul": mmul.astype(bf16),
        "mstat": mstat.astype(bf16),
    }
    in_maps = []
    for c in range(8):
        b, g = c // 4, c % 4
        hs = slice(512 * g, 512 * (g + 1))
        m = dict(shared)
        m["xT"] = np.ascontiguousarray(x[b].T).astype(bf16)
        m["wqT"] = np.ascontiguousarray(wq_s[hs, :].T).astype(bf16)
        m["wkT"] = np.ascontiguousarray(wk_s[hs, :].T).astype(bf16)
        m["wvT"] = np.ascontiguousarray(wv_s[hs, :].T).astype(bf16)
        m["woT"] = np.ascontiguousarray(wo_s[hs, :].T).astype(bf16)
        in_maps.append(m)
    return in_maps


def kernel(x, freqs_cos, freqs_sin, mask, wq, wk, wv, wo):
    in_maps = _host_prep(x, freqs_cos, freqs_sin, wq, wk, wv, wo)
    nc = build_graph()
    results = run_bass_kernel_spmd(nc, in_maps, core_ids=list(range(8))).results
    out = np.empty((B, S, D), np.float32)
    for c in range(8):
        b, g = c // 4, c % 4
        out[b, :, 512 * g : 512 * (g + 1)] = results[c]["out"]
    return out


# revision 6
# speedup vs baseline: 1.6181x; 1.6181x over previous
"""Distributed Trainium2 kernel for causal multi-head attention with RoPE.

Problem (hardcoded): B=2, S=2048, D=2048, H=16, DH=128, float32 I/O.
  out = softmax(mask + rope(x@wq.T) @ rope(x@wk.T).T / sqrt(DH)) @ (x@wv.T) @ wo.T

Sharding over 8 NeuronCores: batch (2) x head-group (4).
Core c handles batch b=c//4 and heads [4g, 4g+4) with g=c%4:
  - QKV projections computed in transposed layout qT/kT [d, tok] (bf16 compute,
    f32 accumulation in PSUM); v in [tok, d] layout.
  - RoPE applied in transposed layout: rot = qT*C + pairswap(qT)*S, where the
    pair swap is a PE matmul with a permutation matrix and C/S are host-built
    [128, 2048] matrices from freqs_cos/sin. 1/sqrt(DH) is folded into wq.
  - Causal attention per head, dual score layouts:
      stats path  [q, k]: exp with accum_out -> row sums -> 1/r broadcast
      AV path     [k, q]: exp tiles (masked) feed attn@V directly, no transpose
    (softmax max-subtraction skipped: scores are O(3), exp cannot overflow)
  - per-head AllGather of attention outputs avT over the 4-core batch group,
    overlapped with the remaining heads' compute.
  - Output projection is column-parallel: each core computes all 2048 tokens of
    its batch for its 512 output columns (woT host-sliced per core).
Host: shards/prepares inputs per core, runs one SPMD NEFF on cores 0-7,
assembles out[b, :, 512g:512(g+1)] from each core.
"""

import sys

for _p in ("/opt/trn_rl_repo", "/root/.axon_site/_ro/trn_rl_repo"):
    if _p not in sys.path:
        sys.path.insert(0, _p)

import math
import numpy as np
import ml_dtypes

import concourse.bass as bass
import concourse.bacc as bacc
import concourse.mybir as mybir
from concourse import tile
from concourse.bass_utils import run_bass_kernel_spmd

bf16 = ml_dtypes.bfloat16
F32 = mybir.dt.float32
F32R = mybir.dt.float32r
BF16 = mybir.dt.bfloat16
Exp = mybir.ActivationFunctionType.Exp
AX = mybir.AxisListType.X
ADD = mybir.AluOpType.add

B, S, D, H = 2, 2048, 2048, 16
DH = D // H  # 128
HPC = 4  # heads per core
GROUPS = [[0, 1, 2, 3], [4, 5, 6, 7]]
NIC = D // 128  # 16 contraction chunks
NTB = S // 512  # 4 token blocks of 512
NTC = S // 128  # 16 token chunks of 128

_GRAPH_CACHE = {}


def build_graph():
    if "nc" in _GRAPH_CACHE:
        return _GRAPH_CACHE["nc"]
    nc = bacc.Bacc(None)

    xT_d = nc.declare_dram_parameter("xT", [D, S], BF16, isOutput=False)
    wqT_d = nc.declare_dram_parameter("wqT", [D, 512], BF16, isOutput=False)
    wkT_d = nc.declare_dram_parameter("wkT", [D, 512], BF16, isOutput=False)
    wvT_d = nc.declare_dram_parameter("wvT", [D, 512], BF16, isOutput=False)
    woT_d = nc.declare_dram_parameter("woT", [D, 512], BF16, isOutput=False)
    cmat_d = nc.declare_dram_parameter("cmat", [128, S], F32, isOutput=False)
    smat_d = nc.declare_dram_parameter("smat", [128, S], F32, isOutput=False)
    pmat_d = nc.declare_dram_parameter("pmat", [128, 128], BF16, isOutput=False)
    mmul_d = nc.declare_dram_parameter("mmul", [128, 2048], BF16, isOutput=False)
    mstat_d = nc.declare_dram_parameter("mstat", [128, 2048], BF16, isOutput=False)
    out_d = nc.declare_dram_parameter("out", [S, 512], F32, isOutput=True)

    ag_in = [nc.dram_tensor(f"ag_in{h}", [128, S], BF16) for h in range(HPC)]
    ag_out = [nc.dram_tensor(f"ag_out{h}", [512, S], BF16) for h in range(HPC)]
    r_bounce = nc.dram_tensor("r_bounce", [HPC, S], F32)

    with tile.TileContext(nc) as tc:
        with tc.tile_pool(name="work", bufs=2) as wk:
            with tc.tile_pool(name="poolA", bufs=1) as pa:
                # persistent across QKV + attention
                qrot = [pa.tile([128, S], BF16, tag=f"q{h}", name=f"qrot{h}") for h in range(HPC)]
                krot = [pa.tile([128, S], BF16, tag=f"k{h}", name=f"krot{h}") for h in range(HPC)]
                vsb = [pa.tile([128, 512], BF16, tag=f"v{j}", name=f"vsb{j}") for j in range(NTC)]

                # ============ Stage 1+2: QKV projections + RoPE =============
                with (
                    tc.tile_pool(name="qkvw", bufs=1) as qw,
                    tc.tile_pool(name="psq", bufs=4, space="PSUM") as psq,
                    tc.tile_pool(name="pssh", bufs=2, space="PSUM") as pssh,
                    tc.tile_pool(name="psv", bufs=2, space="PSUM") as psv,
                ):
                    xt = [qw.tile([128, S], BF16, tag=f"xt{i}", name=f"xt{i}") for i in range(NIC)]
                    wq_sb = [qw.tile([128, 512], BF16, tag=f"wq{i}", name=f"wqsb{i}") for i in range(NIC)]
                    wk_sb = [qw.tile([128, 512], BF16, tag=f"wk{i}", name=f"wksb{i}") for i in range(NIC)]
                    for i in range(NIC):
                        nc.sync.dma_start(wq_sb[i][:], wqT_d[128 * i : 128 * (i + 1), :])
                        nc.sync.dma_start(wk_sb[i][:], wkT_d[128 * i : 128 * (i + 1), :])
                        nc.sync.dma_start(xt[i][:], xT_d[128 * i : 128 * (i + 1), :])
                    cs_sb = qw.tile([128, S], F32, tag="cs")
                    sn_sb = qw.tile([128, S], F32, tag="sn")
                    pmat_sb = qw.tile([128, 128], BF16, tag="pmat")
                    nc.sync.dma_start(cs_sb[:], cmat_d[:])
                    nc.sync.dma_start(sn_sb[:], smat_d[:])
                    nc.sync.dma_start(pmat_sb[:], pmat_d[:])
                    wv_sb = [qw.tile([128, 512], BF16, tag=f"wv{i}", name=f"wvsb{i}") for i in range(NIC)]
                    for i in range(NIC):
                        nc.sync.dma_start(wv_sb[i][:], wvT_d[128 * i : 128 * (i + 1), :])

                    # Q and K projections -> transposed layout [d, tok] + RoPE
                    for w_sb, rot in ((wq_sb, qrot), (wk_sb, krot)):
                        for h in range(HPC):
                            for b in range(NTB):
                                ps = psq.tile([128, 512], F32, tag="qk")
                                for i in range(NIC):
                                    nc.tensor.matmul(
                                        ps[:],
                                        w_sb[i][:, 128 * h : 128 * (h + 1)],
                                        xt[i][:, 512 * b : 512 * (b + 1)],
                                        start=(i == 0),
                                        stop=(i == NIC - 1),
                                    )
                                raw = wk.tile([128, 512], BF16, tag="raw")
                                nc.scalar.copy(raw[:], ps[:])
                                shp = pssh.tile([128, 512], F32, tag="sh")
                                nc.tensor.matmul(shp[:], pmat_sb[:], raw[:])
                                t1 = wk.tile([128, 512], F32, tag="t1")
                                t2 = wk.tile([128, 512], F32, tag="t2")
                                nc.vector.tensor_mul(t1[:], ps[:], cs_sb[:, 512 * b : 512 * (b + 1)])
                                nc.vector.tensor_mul(t2[:], shp[:], sn_sb[:, 512 * b : 512 * (b + 1)])
                                nc.vector.tensor_add(rot[h][:, 512 * b : 512 * (b + 1)], t1[:], t2[:])

                    # V projection -> [tok, d] layout
                    for j in range(NTC):
                        ps = psv.tile([128, 512], F32, tag="v")
                        for i in range(NIC):
                            nc.tensor.matmul(
                                ps[:],
                                xt[i][:, 128 * j : 128 * (j + 1)],
                                wv_sb[i][:],
                                start=(i == 0),
                                stop=(i == NIC - 1),
                            )
                        nc.scalar.copy(vsb[j][:], ps[:])

                # wo weights loaded early (independent of attention/collective)
                with tc.tile_pool(name="wosb", bufs=1) as wop:
                    wo_sb = [wop.tile([128, 512], BF16, tag=f"wo{cc}", name=f"wosb{cc}") for cc in range(NIC)]
                    for cc in range(NIC):
                        nc.sync.dma_start(wo_sb[cc][:], woT_d[128 * cc : 128 * (cc + 1), :])

                    # ============ Stage 3: attention per head ===============
                    with (
                        tc.tile_pool(name="attn", bufs=2) as at,
                        tc.tile_pool(name="attn1", bufs=1) as at1,
                        tc.tile_pool(name="psa", bufs=2, space="PSUM") as psa,
                        tc.tile_pool(name="psb", bufs=2, space="PSUM") as psb,
                        tc.tile_pool(name="psav", bufs=2, space="PSUM") as psav,
                        tc.tile_pool(name="psr", bufs=2, space="PSUM") as psr,
                    ):
                        avsb = [at1.tile([128, S], BF16, tag=f"av{h}", name=f"avsb{h}") for h in range(HPC)]
                        mmul_sb = at1.tile([128, 2048], BF16, tag="mmul")
                        mstat_sb = at1.tile([128, 2048], BF16, tag="mstat")
                        ones_sb = at1.tile([1, 128], F32, tag="ones")
                        nc.vector.memset(ones_sb[:], 1.0)
                        nc.sync.dma_start(mmul_sb[:], mmul_d[:])
                        nc.sync.dma_start(mstat_sb[:], mstat_d[:])

                        for h in range(HPC):
                            # ---- stats: r[q] = sum over valid k of exp ----
                            rall = at.tile([128, NTC], F32, tag="rall")
                            for qb in range(NTC):
                                q0 = 128 * qb
                                nk = qb // 4 + 1
                                racc = wk.tile([128, 4], F32, tag="racc")
                                for j in range(nk):
                                    w = 128 * (qb % 4) + 128 if j == nk - 1 else 512
                                    ps = psa.tile([128, 512], F32, tag="sa")
                                    nc.tensor.matmul(
                                        ps[:, :w],
                                        qrot[h][:, q0 : q0 + 128],
                                        krot[h][:, 512 * j : 512 * j + w],
                                    )
                                    es = wk.tile([128, 512], BF16, tag="es")
                                    if j == nk - 1:
                                        # diagonal-band tile: exp, mask, reduce
                                        nc.scalar.activation(es[:, :w], ps[:, :w], Exp)
                                        em = wk.tile([128, 512], BF16, tag="em")
                                        moff = 512 * (qb % 4)
                                        nc.vector.tensor_mul(em[:, :w], es[:, :w], mstat_sb[:, moff : moff + w])
                                        nc.vector.tensor_reduce(racc[:, j : j + 1], em[:, :w], AX, ADD)
                                    else:
                                        nc.scalar.activation(es[:], ps[:], Exp, accum_out=racc[:, j : j + 1])
                                nc.vector.tensor_reduce(rall[:, qb : qb + 1], racc[:, :nk], AX, ADD)
                            # 1/r -> [1, S] via DRAM bounce (in flight under AV)
                            rinv = at.tile([128, NTC], F32, tag="rinv")
                            nc.vector.reciprocal(rinv[:], rall[:])
                            nc.sync.dma_start(
                                r_bounce[h].rearrange("(b a) -> a b", a=128, b=NTC), rinv[:]
                            )
                            rT = at.tile([1, S], F32, tag="rT")
                            nc.sync.dma_start(rT[0:1, :], r_bounce[h][None, :])
                            rbc = at.tile([128, S], F32, tag="rbc")

                            # ---- AV path: avT[d, q] accumulation over k ----
                            for b in range(NTB):
                                q0 = 512 * b
                                nk2 = 4 * (b + 1)
                                av = psav.tile([128, 512], F32, tag="av")
                                for kc in range(nk2):
                                    j = kc - 4 * b  # >= 0 on the diagonal band
                                    o = 128 * j if j > 0 else 0
                                    w = 512 - o
                                    ps = psb.tile([128, 512], F32, tag="sb")
                                    nc.tensor.matmul(
                                        ps[:, :w],
                                        krot[h][:, 128 * kc : 128 * (kc + 1)],
                                        qrot[h][:, q0 + o : q0 + 512],
                                    )
                                    et = at.tile([128, 512], BF16, tag="et")
                                    nc.scalar.activation(et[:, :w], ps[:, :w], Exp)
                                    if j >= 0:
                                        nc.vector.tensor_mul(et[:, :w], et[:, :w], mmul_sb[:, :w])
                                    nc.tensor.matmul(
                                        av[:, o:512],
                                        vsb[kc][:, 128 * h : 128 * (h + 1)],
                                        et[:, :w],
                                        start=(kc == 0),
                                        stop=(kc == nk2 - 1),
                                    )
                                # broadcast 1/r rows (f32r: full-rate PE) + normalize
                                psn = psr.tile([128, 512], F32, tag="rb")
                                nc.tensor.matmul(
                                    psn[:],
                                    ones_sb[:].bitcast(F32R),
                                    rT[0:1, q0 : q0 + 512].bitcast(F32R),
                                )
                                nc.scalar.copy(rbc[:, q0 : q0 + 512], psn[:])
                                nc.vector.tensor_mul(
                                    avsb[h][:, q0 : q0 + 512], av[:], rbc[:, q0 : q0 + 512]
                                )

                            # ship this head to the batch group immediately
                            nc.sync.dma_start(ag_in[h][:], avsb[h][:])
                            nc.gpsimd.collective_compute(
                                "AllGather",
                                mybir.AluOpType.bypass,
                                replica_groups=GROUPS,
                                ins=[ag_in[h][:]],
                                outs=[ag_out[h][:]],
                            )

                    # ============ Stage 4: column-parallel wo projection ====
                    with (
                        tc.tile_pool(name="agp", bufs=1) as agp,
                        tc.tile_pool(name="pswo", bufs=4, space="PSUM") as pswo,
                    ):
                        ag_sb = [agp.tile([128, S], BF16, tag=f"ag{g}", name=f"agsb{g}") for g in range(NIC)]
                        for g in range(NIC):
                            h, r = g % 4, g // 4
                            nc.sync.dma_start(ag_sb[g][:], ag_out[h][128 * r : 128 * (r + 1), :])
                        for t in range(NTC):
                            ps = pswo.tile([128, 512], F32, tag="wo")
                            for cc in range(NIC):
                                nc.tensor.matmul(
                                    ps[:],
                                    ag_sb[cc][:, 128 * t : 128 * (t + 1)],
                                    wo_sb[cc][:],
                                    start=(cc == 0),
                                    stop=(cc == NIC - 1),
                                )
                            osb = agp.tile([128, 512], F32, tag="osb", bufs=2)
                            nc.vector.tensor_copy(osb[:], ps[:])
                            nc.sync.dma_start(out_d[128 * t : 128 * (t + 1), :], osb[:])

    nc.finalize()
    _GRAPH_CACHE["nc"] = nc
    return nc


def _host_prep(x, freqs_cos, freqs_sin, wq, wk, wv, wo):
    """Build the 8 per-core input maps."""
    fc = np.asarray(freqs_cos, np.float32)  # [S, 64]
    fs = np.asarray(freqs_sin, np.float32)
    cmat = np.empty((128, S), np.float32)
    smat = np.empty((128, S), np.float32)
    cmat[0::2, :] = fc.T[:, :]  # row 2i   <- cos[:, i]
    cmat[1::2, :] = fc.T[:, :]
    smat[0::2, :] = -fs.T[:, :]  # rot[2i]   = a*c - b*s ; shuf[2i]   = b
    smat[1::2, :] = fs.T[:, :]  # rot[2i+1] = b*c + a*s ; shuf[2i+1] = a
    pmat = np.zeros((128, 128), np.float32)
    for i in range(64):
        pmat[2 * i, 2 * i + 1] = 1.0  # shuf = P @ q, P symmetric pair swap
        pmat[2 * i + 1, 2 * i] = 1.0

    xs = np.arange(128)[:, None]
    ys = np.arange(512)[None, :]
    # AV-path masks, [128 k x 512 q] tiles: valid iff x + 128*j <= y
    mm = np.zeros((128, 4, 512), np.float32)
    # stats-path masks, [128 q x 512 k] tiles: valid iff y <= x + 128*o
    ms = np.zeros((128, 4, 512), np.float32)
    for j in range(4):
        mm[:, j, :] = (xs + 128 * j <= ys).astype(np.float32)
        ms[:, j, :] = (ys <= xs + 128 * j).astype(np.float32)
    mmul = mm.reshape(128, 2048)
    mstat = ms.reshape(128, 2048)

    wq_s = np.asarray(wq, np.float32) / math.sqrt(DH)
    wk_s = np.asarray(wk, np.float32)
    wv_s = np.asarray(wv, np.float32)
    wo_s = np.asarray(wo, np.float32)
    x = np.asarray(x, np.float32)

    shared = {
        "cmat": cmat,
        "smat": smat,
        "pmat": pmat.astype(bf16),
        "mmul": mmul.astype(bf16),
        "mstat": mstat.astype(bf16),
    }
    in_maps = []
    for c in range(8):
        b, g = c // 4, c % 4
        hs = slice(512 * g, 512 * (g + 1))
        m = dict(shared)
        m["xT"] = np.ascontiguousarray(x[b].T).astype(bf16)
        m["wqT"] = np.ascontiguousarray(wq_s[hs, :].T).astype(bf16)
        m["wkT"] = np.ascontiguousarray(wk_s[hs, :].T).astype(bf16)
        m["wvT"] = np.ascontiguousarray(wv_s[hs, :].T).astype(bf16)
        m["woT"] = np.ascontiguousarray(wo_s[hs, :].T).astype(bf16)
        in_maps.append(m)
    return in_maps


def kernel(x, freqs_cos, freqs_sin, mask, wq, wk, wv, wo):
    in_maps = _host_prep(x, freqs_cos, freqs_sin, wq, wk, wv, wo)
    nc = build_graph()
    results = run_bass_kernel_spmd(nc, in_maps, core_ids=list(range(8))).results
    out = np.empty((B, S, D), np.float32)
    for c in range(8):
        b, g = c // 4, c % 4
        out[b, :, 512 * g : 512 * (g + 1)] = results[c]["out"]
    return out


# revision 9
# speedup vs baseline: 1.7736x; 1.0961x over previous
"""Distributed Trainium2 kernel for causal multi-head attention with RoPE.

Problem (hardcoded): B=2, S=2048, D=2048, H=16, DH=128, float32 I/O.
  out = softmax(mask + rope(x@wq.T) @ rope(x@wk.T).T / sqrt(DH)) @ (x@wv.T) @ wo.T

Sharding over 8 NeuronCores: batch (2) x head-group (4).
Core c handles batch b=c//4 and heads [4g, 4g+4) with g=c%4:
  - QKV projections computed in transposed layout qT/kT [d, tok] (bf16 compute,
    f32 accumulation in PSUM); v in [tok, d] layout.
  - RoPE applied in transposed layout: rot = qT*C + pairswap(qT)*S, where the
    pair swap is a PE matmul with a permutation matrix and C/S are host-built
    [128, 2048] matrices from freqs_cos/sin. 1/sqrt(DH) is folded into wq.
  - Causal attention per head in transposed score layout [k, q]: masked exp
    tiles feed both attn@V and a ones-row matmul that accumulates the softmax
    denominators in [1, q] row layout (no max-subtraction: scores are O(3)).
    Normalization multiplies by a PE-broadcast of 1/r.
  - Per-head 8-way AllToAll ships each core's heads to the group peer that owns
    the destination token block (cross-batch shards are duplicates, selected
    away at receive time with per-core 0/1 scalars).
  - Output projection is token-parallel: each core computes its 512 tokens for
    all 2048 output columns with the full wo.
Host: shards/prepares inputs per core, runs one SPMD NEFF on cores 0-7,
assembles out[b, 512g:512(g+1), :] from each core.
"""

import sys

for _p in ("/opt/trn_rl_repo", "/root/.axon_site/_ro/trn_rl_repo"):
    if _p not in sys.path:
        sys.path.insert(0, _p)

import math
import numpy as np
import ml_dtypes

import concourse.bass as bass
import concourse.bacc as bacc
import concourse.mybir as mybir
from concourse import tile
from concourse.bass_utils import run_bass_kernel_spmd

bf16 = ml_dtypes.bfloat16
F32 = mybir.dt.float32
F32R = mybir.dt.float32r
BF16 = mybir.dt.bfloat16
Exp = mybir.ActivationFunctionType.Exp
AX = mybir.AxisListType.X
ADD = mybir.AluOpType.add

B, S, D, H = 2, 2048, 2048, 16
DH = D // H  # 128
HPC = 4  # heads per core
GROUPS = [[0, 1, 2, 3, 4, 5, 6, 7]]
NIC = D // 128  # 16 contraction chunks
NTB = S // 512  # 4 token blocks of 512
NTC = S // 128  # 16 token chunks of 128

_GRAPH_CACHE = {}


def build_graph():
    if "nc" in _GRAPH_CACHE:
        return _GRAPH_CACHE["nc"]
    nc = bacc.Bacc(None)

    xT_d = nc.declare_dram_parameter("xT", [D, S], BF16, isOutput=False)
    wqT_d = nc.declare_dram_parameter("wqT", [D, 512], BF16, isOutput=False)
    wkT_d = nc.declare_dram_parameter("wkT", [D, 512], BF16, isOutput=False)
    wvT_d = nc.declare_dram_parameter("wvT", [D, 512], BF16, isOutput=False)
    woT_d = nc.declare_dram_parameter("woT", [D, D], BF16, isOutput=False)
    cmat_d = nc.declare_dram_parameter("cmat", [128, S], F32, isOutput=False)
    smat_d = nc.declare_dram_parameter("smat", [128, S], F32, isOutput=False)
    pmat_d = nc.declare_dram_parameter("pmat", [128, 128], BF16, isOutput=False)
    mmul_d = nc.declare_dram_parameter("mmul", [128, 512], BF16, isOutput=False)
    gsel_d = nc.declare_dram_parameter("gsel", [128, 2], F32, isOutput=False)
    out_d = nc.declare_dram_parameter("out", [512, D], F32, isOutput=True)

    a2a_in = [nc.dram_tensor(f"a2a_in{h}", [1024, 512], BF16) for h in range(HPC)]
    a2a_out = [nc.dram_tensor(f"a2a_out{h}", [1024, 512], BF16) for h in range(HPC)]

    with tile.TileContext(nc) as tc:
        with tc.tile_pool(name="work", bufs=2) as wk:
            with tc.tile_pool(name="poolA", bufs=1) as pa:
                # persistent across QKV + attention
                qrot = [pa.tile([128, S], BF16, tag=f"q{h}", name=f"qrot{h}") for h in range(HPC)]
                krot = [pa.tile([128, S], BF16, tag=f"k{h}", name=f"krot{h}") for h in range(HPC)]
                vsb = [pa.tile([128, 512], BF16, tag=f"v{j}", name=f"vsb{j}") for j in range(NTC)]

                # ============ Stage 1+2: QKV projections + RoPE =============
                with (
                    tc.tile_pool(name="qkvw", bufs=1) as qw,
                    tc.tile_pool(name="psq", bufs=4, space="PSUM") as psq,
                    tc.tile_pool(name="pssh", bufs=2, space="PSUM") as pssh,
                    tc.tile_pool(name="psv", bufs=2, space="PSUM") as psv,
                ):
                    xt = [qw.tile([128, S], BF16, tag=f"xt{i}", name=f"xt{i}") for i in range(NIC)]
                    wq_sb = [qw.tile([128, 512], BF16, tag=f"wq{i}", name=f"wqsb{i}") for i in range(NIC)]
                    wk_sb = [qw.tile([128, 512], BF16, tag=f"wk{i}", name=f"wksb{i}") for i in range(NIC)]
                    for i in range(NIC):
                        nc.sync.dma_start(wq_sb[i][:], wqT_d[128 * i : 128 * (i + 1), :])
                        nc.sync.dma_start(wk_sb[i][:], wkT_d[128 * i : 128 * (i + 1), :])
                        nc.sync.dma_start(xt[i][:], xT_d[128 * i : 128 * (i + 1), :])
                    cs_sb = qw.tile([128, S], F32, tag="cs")
                    sn_sb = qw.tile([128, S], F32, tag="sn")
                    pmat_sb = qw.tile([128, 128], BF16, tag="pmat")
                    nc.sync.dma_start(cs_sb[:], cmat_d[:])
                    nc.sync.dma_start(sn_sb[:], smat_d[:])
                    nc.sync.dma_start(pmat_sb[:], pmat_d[:])
                    wv_sb = [qw.tile([128, 512], BF16, tag=f"wv{i}", name=f"wvsb{i}") for i in range(NIC)]
                    for i in range(NIC):
                        nc.sync.dma_start(wv_sb[i][:], wvT_d[128 * i : 128 * (i + 1), :])

                    # Q and K projections -> transposed layout [d, tok] + RoPE
                    for w_sb, rot in ((wq_sb, qrot), (wk_sb, krot)):
                        for h in range(HPC):
                            for b in range(NTB):
                                ps = psq.tile([128, 512], F32, tag="qk")
                                for i in range(NIC):
                                    nc.tensor.matmul(
                                        ps[:],
                                        w_sb[i][:, 128 * h : 128 * (h + 1)],
                                        xt[i][:, 512 * b : 512 * (b + 1)],
                                        start=(i == 0),
                                        stop=(i == NIC - 1),
                                    )
                                raw = wk.tile([128, 512], BF16, tag="raw")
                                nc.scalar.copy(raw[:], ps[:])
                                shp = pssh.tile([128, 512], F32, tag="sh")
                                nc.tensor.matmul(shp[:], pmat_sb[:], raw[:])
                                t1 = wk.tile([128, 512], F32, tag="t1")
                                t2 = wk.tile([128, 512], F32, tag="t2")
                                nc.vector.tensor_mul(t1[:], ps[:], cs_sb[:, 512 * b : 512 * (b + 1)])
                                nc.vector.tensor_mul(t2[:], shp[:], sn_sb[:, 512 * b : 512 * (b + 1)])
                                nc.vector.tensor_add(rot[h][:, 512 * b : 512 * (b + 1)], t1[:], t2[:])

                    # V projection -> [tok, d] layout
                    for j in range(NTC):
                        ps = psv.tile([128, 512], F32, tag="v")
                        for i in range(NIC):
                            nc.tensor.matmul(
                                ps[:],
                                xt[i][:, 128 * j : 128 * (j + 1)],
                                wv_sb[i][:],
                                start=(i == 0),
                                stop=(i == NIC - 1),
                            )
                        nc.scalar.copy(vsb[j][:], ps[:])

                # wo weights loaded early (independent of attention/collective)
                with tc.tile_pool(name="wosb", bufs=1) as wop:
                    wo_sb = [wop.tile([128, D], BF16, tag=f"wo{cc}", name=f"wosb{cc}") for cc in range(NIC)]
                    for cc in range(NIC):
                        nc.sync.dma_start(wo_sb[cc][:], woT_d[128 * cc : 128 * (cc + 1), :])

                    # ============ Stage 3: attention per head ===============
                    with (
                        tc.tile_pool(name="attn", bufs=3) as at,
                        tc.tile_pool(name="attn1", bufs=1) as at1,
                        tc.tile_pool(name="psb", bufs=2, space="PSUM") as psb,
                        tc.tile_pool(name="psav", bufs=2, space="PSUM") as psav,
                        tc.tile_pool(name="psrs", bufs=2, space="PSUM") as psrs,
                        tc.tile_pool(name="psr", bufs=2, space="PSUM") as psr,
                    ):
                        mmul_sb = at1.tile([128, 512], BF16, tag="mmul")
                        ones_bf = at1.tile([128, 1], BF16, tag="ones_bf")
                        ones_f32 = at1.tile([1, 128], F32, tag="ones_f32")
                        ones_r = at1.tile([1, 128], F32R, tag="ones_r")
                        nc.vector.memset(ones_bf[:], 1.0)
                        nc.vector.memset(ones_f32[:], 1.0)
                        nc.vector.tensor_copy(ones_r[:], ones_f32[:])
                        nc.sync.dma_start(mmul_sb[:], mmul_d[:])

                        for h in range(HPC):
                            for b in range(NTB):
                                q0 = 512 * b
                                nk2 = 4 * (b + 1)
                                av = psav.tile([128, 512], F32, tag="av")
                                rsum = psrs.tile([1, 512], F32, tag="rs")
                                for kc in range(nk2):
                                    j = kc - 4 * b  # >= 0 on the diagonal band
                                    o = 128 * j if j > 0 else 0
                                    w = 512 - o
                                    ps = psb.tile([128, 512], F32, tag="sb")
                                    nc.tensor.matmul(
                                        ps[:, :w],
                                        krot[h][:, 128 * kc : 128 * (kc + 1)],
                                        qrot[h][:, q0 + o : q0 + 512],
                                    )
                                    et = at.tile([128, 512], BF16, tag="et")
                                    nc.scalar.activation(et[:, :w], ps[:, :w], Exp)
                                    if j >= 0:
                                        nc.vector.tensor_mul(et[:, :w], et[:, :w], mmul_sb[:, :w])
                                    nc.tensor.matmul(
                                        av[:, o:512],
                                        vsb[kc][:, 128 * h : 128 * (h + 1)],
                                        et[:, :w],
                                        start=(kc == 0),
                                        stop=(kc == nk2 - 1),
                                    )
                                    nc.tensor.matmul(
                                        rsum[0:1, o:512],
                                        ones_bf[:],
                                        et[:, :w],
                                        start=(kc == 0),
                                        stop=(kc == nk2 - 1),
                                    )
                                # denominators: 1/r broadcast to 128 partitions
                                rsC = wk.tile([1, 512], F32, tag="rsC")
                                nc.scalar.copy(rsC[:], rsum[:])
                                rinv = wk.tile([1, 512], F32, tag="rinv")
                                nc.vector.reciprocal(rinv[:], rsC[:])
                                rinv_r = wk.tile([1, 512], F32R, tag="rinv_r")
                                nc.vector.tensor_copy(rinv_r[:], rinv[:])
                                psn = psr.tile([128, 512], F32, tag="rb")
                                nc.tensor.matmul(
                                    psn[:],
                                    ones_r[:],
                                    rinv_r[:],
                                )
                                rbc = wk.tile([128, 512], F32, tag="rbc")
                                nc.scalar.copy(rbc[:], psn[:])
                                avn = at.tile([128, 512], BF16, tag="avn")
                                nc.vector.tensor_mul(avn[:], av[:], rbc[:])
                                # ship token block b of head h to both owner candidates
                                nc.sync.dma_start(a2a_in[h][128 * b : 128 * (b + 1), :], avn[:])
                                nc.sync.dma_start(a2a_in[h][512 + 128 * b : 512 + 128 * (b + 1), :], avn[:])
                            nc.gpsimd.collective_compute(
                                "AllToAll",
                                mybir.AluOpType.bypass,
                                replica_groups=GROUPS,
                                ins=[a2a_in[h][:]],
                                outs=[a2a_out[h][:]],
                            )

                    # ============ Stage 4: token-parallel wo projection =====
                    with (
                        tc.tile_pool(name="agp", bufs=1) as agp,
                        tc.tile_pool(name="agw", bufs=3) as agw,
                        tc.tile_pool(name="pswo", bufs=4, space="PSUM") as pswo,
                    ):
                        gsel_sb = agp.tile([128, 2], F32, tag="gsel")
                        nc.sync.dma_start(gsel_sb[:], gsel_d[:])
                        agc = [agp.tile([128, 512], BF16, tag=f"agc{g}", name=f"agc{g}") for g in range(NIC)]
                        for g in range(NIC):
                            h, r = g % 4, g // 4
                            lo = agw.tile([128, 512], BF16, tag="lo")
                            hi = agw.tile([128, 512], BF16, tag="hi")
                            nc.sync.dma_start(lo[:], a2a_out[h][128 * r : 128 * (r + 1), :])
                            nc.sync.dma_start(hi[:], a2a_out[h][512 + 128 * r : 512 + 128 * (r + 1), :])
                            c1 = agw.tile([128, 512], BF16, tag="c1")
                            nc.vector.tensor_scalar_mul(c1[:], lo[:], gsel_sb[:, 0:1])
                            c2 = agw.tile([128, 512], BF16, tag="c2")
                            nc.vector.tensor_scalar_mul(c2[:], hi[:], gsel_sb[:, 1:2])
                            nc.vector.tensor_add(agc[g][:], c1[:], c2[:])
                        for t in range(4):
                            osb = agp.tile([128, D], F32, tag="osb", bufs=2)
                            for oc in range(4):
                                ps = pswo.tile([128, 512], F32, tag="wo")
                                for g in range(NIC):
                                    nc.tensor.matmul(
                                        ps[:],
                                        agc[g][:, 128 * t : 128 * (t + 1)],
                                        wo_sb[g][:, 512 * oc : 512 * (oc + 1)],
                                        start=(g == 0),
                                        stop=(g == NIC - 1),
                                    )
                                nc.vector.tensor_copy(osb[:, 512 * oc : 512 * (oc + 1)], ps[:])
                            nc.sync.dma_start(out_d[128 * t : 128 * (t + 1), :], osb[:])

    nc.finalize()
    _GRAPH_CACHE["nc"] = nc
    return nc


def _host_prep(x, freqs_cos, freqs_sin, wq, wk, wv, wo):
    """Build the 8 per-core input maps."""
    fc = np.asarray(freqs_cos, np.float32)  # [S, 64]
    fs = np.asarray(freqs_sin, np.float32)
    cmat = np.empty((128, S), np.float32)
    smat = np.empty((128, S), np.float32)
    cmat[0::2, :] = fc.T[:, :]  # row 2i   <- cos[:, i]
    cmat[1::2, :] = fc.T[:, :]
    smat[0::2, :] = -fs.T[:, :]  # rot[2i]   = a*c - b*s ; shuf[2i]   = b
    smat[1::2, :] = fs.T[:, :]  # rot[2i+1] = b*c + a*s ; shuf[2i+1] = a
    pmat = np.zeros((128, 128), np.float32)
    for i in range(64):
        pmat[2 * i, 2 * i + 1] = 1.0  # shuf = P @ q, P symmetric pair swap
        pmat[2 * i + 1, 2 * i] = 1.0

    xs = np.arange(128)[:, None]
    ys = np.arange(512)[None, :]
    # AV-path mask for [128 k x 512 q] diagonal tiles: valid iff x <= y
    mmul = (xs <= ys).astype(np.float32)

    wq_s = np.asarray(wq, np.float32) / math.sqrt(DH)
    wk_s = np.asarray(wk, np.float32)
    wv_s = np.asarray(wv, np.float32)
    woT = np.ascontiguousarray(np.asarray(wo, np.float32).T).astype(bf16)
    x = np.asarray(x, np.float32)

    shared = {
        "cmat": cmat,
        "smat": smat,
        "pmat": pmat.astype(bf16),
        "mmul": mmul.astype(bf16),
        "woT": woT,
    }
    in_maps = []
    for c in range(8):
        b, g = c // 4, c % 4
        hs = slice(512 * g, 512 * (g + 1))
        m = dict(shared)
        m["xT"] = np.ascontiguousarray(x[b].T).astype(bf16)
        m["wqT"] = np.ascontiguousarray(wq_s[hs, :].T).astype(bf16)
        m["wkT"] = np.ascontiguousarray(wk_s[hs, :].T).astype(bf16)
        m["wvT"] = np.ascontiguousarray(wv_s[hs, :].T).astype(bf16)
        gsel = np.zeros((128, 2), np.float32)
        gsel[:, b] = 1.0
        m["gsel"] = gsel
        in_maps.append(m)
    return in_maps


def kernel(x, freqs_cos, freqs_sin, mask, wq, wk, wv, wo):
    in_maps = _host_prep(x, freqs_cos, freqs_sin, wq, wk, wv, wo)
    nc = build_graph()
    results = run_bass_kernel_spmd(nc, in_maps, core_ids=list(range(8))).results
    out = np.empty((B, S, D), np.float32)
    for c in range(8):
        b, g = c // 4, c % 4
        out[b, 512 * g : 512 * (g + 1), :] = results[c]["out"]
    return out
